# revision 1
# baseline (speedup 1.0000x reference)
"""Multi-head attention (B=4, S=2048, D=1024, H=16, HD=64) on 8 TRN2 NeuronCores.

Sharding: core c handles batch b=c//2 and head-group g=c%2 (8 heads).
W_q/W_k/W_v column-sharded, W_o row-sharded; the two partial outputs per
batch are summed on the host (cheap add, avoids on-device collectives).

Per-core kernel (all matmuls bf16, accumulation fp32 in PSUM):
  1. Projections.  Host supplies x^T [D, S] bf16, so TensorE can contract
     over D directly:  qT/kT [128, S] per head-pair (two heads stacked on
     the partition dim), v [S, 8*65] with a ones column appended per head.
  2. Attention in transposed layout: scoresT[j, i] = k q^T computed with
     two row-tiled (K=64) matmuls per j-tile — one per head of the pair —
     into one PSUM tile; exp on ScalarE (scale=1/8 folded in); causal
     structure is derived from the actual mask at build time: fully-masked
     128x128 blocks are skipped, mixed blocks are multiplied by a 0/1
     valid matrix, fully-valid blocks are untouched.
  3. ctx^T and the softmax denominator come from one M=65 matmul per
     (head, j-tile): lhsT = [v | 1] so PSUM row 64 accumulates sum(attn).
     Normalization: vector reciprocal of row 64, broadcast via a DRAM
     bounce DMA, one tensor_mul.  Head-1 results are DMA-shifted to
     partitions 64..127 to form pair-stacked ctxT [128, S].
  4. Output projection: out[s, :] += ctxT_pair.T @ W_o rows, K=128 per
     pair, accumulated over the 4 pairs.
"""

import sys

sys.path.insert(0, "/opt/trn_rl_repo")

import numpy as np
import ml_dtypes

import concourse.bacc as bacc
import concourse.tile as tile
from concourse import mybir

BF16 = ml_dtypes.bfloat16
F32 = mybir.dt.float32
BF = mybir.dt.bfloat16

B, S, D, H, HD = 4, 2048, 1024, 16, 64
G = 2              # head groups (cores per batch)
HPG = H // G       # 8 heads per group
NPAIR = HPG // 2   # 4 head pairs
FB = HPG * HD      # 512 projection cols per group
BLK = 128          # j-tile size
IBW = 512          # i-block width
NIB = S // IBW     # 4 i-blocks
NJT = S // BLK     # 16 j-tiles
NDT = D // BLK     # 8 contraction tiles
NST = S // BLK     # 16 s-tiles for the output projection
VW = HD + 1        # 65: v plus ones column
EXP_SCALE = 1.0 / np.sqrt(np.float32(HD))


def classify_mask(mask: np.ndarray):
    """Block states over the *transposed* mask grid: state[jt][it] for the
    region j in [128jt,..), i in [128it,..).  0=all valid, 1=all masked,
    2=mixed."""
    m = np.asarray(mask)
    blocks = m.reshape(NJT, BLK, NJT, BLK).transpose(0, 2, 1, 3)  # [it, jt, i, j]
    anym = blocks.any(axis=(2, 3))
    allm = blocks.all(axis=(2, 3))
    states = np.where(allm, 1, np.where(anym, 2, 0)).astype(np.int8)
    return states.T  # index [jt, it]


def build_plan(states: np.ndarray):
    """Per i-block: list of (jt, c0, c1, mixed_ks).  c0/c1 bound the valid
    i-columns (relative to the block) at 128 granularity; mixed_ks are the
    128-col sub-blocks needing a 0/1 multiply (includes interior all-masked
    sub-blocks, which get an all-zero valid matrix)."""
    plan = []
    mixed_slots = {}
    for ib in range(NIB):
        its = list(range(4 * ib, 4 * ib + 4))
        jts = []
        for jt in range(NJT):
            sub = [int(states[jt, it]) for it in its]
            nz = [k for k, st in enumerate(sub) if st != 1]
            if not nz:
                continue
            k0, k1 = nz[0], nz[-1]
            mixed = [k for k in range(k0, k1 + 1) if sub[k] != 0]
            for k in mixed:
                mixed_slots.setdefault((jt, its[k]), len(mixed_slots))
            jts.append((jt, k0 * BLK, (k1 + 1) * BLK, mixed))
        assert jts, "fully-masked i-block not supported"
        plan.append(jts)
    return plan, mixed_slots


def plan_key(plan, mixed_slots):
    return (
        tuple(
            tuple((jt, c0, c1, tuple(mk)) for jt, c0, c1, mk in jts) for jts in plan
        ),
        tuple(sorted(mixed_slots.items())),
    )


def build_nc(plan, mixed_slots):
    nvb = max(1, len(mixed_slots))
    nc = bacc.Bacc("TRN2", target_bir_lowering=False, debug=False, num_devices=8)

    xqT = nc.dram_tensor("xqT", [D, S], BF, kind="ExternalInput").ap()
    xkT = nc.dram_tensor("xkT", [D, S], BF, kind="ExternalInput").ap()
    xvT = nc.dram_tensor("xvT", [D, S], BF, kind="ExternalInput").ap()
    wq = nc.dram_tensor("wq", [D, FB], BF, kind="ExternalInput").ap()
    wk = nc.dram_tensor("wk", [D, FB], BF, kind="ExternalInput").ap()
    wv = nc.dram_tensor("wv", [D, FB], BF, kind="ExternalInput").ap()
    wo = nc.dram_tensor("wo", [FB, D], BF, kind="ExternalInput").ap()
    validT = nc.dram_tensor("validT", [nvb, BLK, BLK], BF, kind="ExternalInput").ap()
    out = nc.dram_tensor("out", [S, D], F32, kind="ExternalOutput").ap()

    with tile.TileContext(nc) as tc:
        import contextlib

        ctxmgr = contextlib.ExitStack()
        with ctxmgr:
            persist = ctxmgr.enter_context(tc.tile_pool(name="persist", bufs=1))
            xpool = ctxmgr.enter_context(tc.tile_pool(name="xpool", bufs=16))
            scp = ctxmgr.enter_context(tc.tile_pool(name="scp", bufs=2, space="PSUM"))
            projp = ctxmgr.enter_context(tc.tile_pool(name="projp", bufs=2, space="PSUM"))
            ctxp = ctxmgr.enter_context(tc.tile_pool(name="ctxp", bufs=2, space="PSUM"))
            atp = ctxmgr.enter_context(tc.tile_pool(name="atp", bufs=3))
            small = ctxmgr.enter_context(tc.tile_pool(name="small", bufs=4))
            drp = ctxmgr.enter_context(tc.tile_pool(name="drp", bufs=4, space="DRAM"))

            # ---- persistent weights / activations -------------------------
            # DMA order matters: v-projection inputs first so PE starts
            # early; spread loads over two queues (sync + gpsimd).
            wv_t = [persist.tile([BLK, FB], BF, name=f"wv{d}") for d in range(NDT)]
            xv_t = [xpool.tile([BLK, S], BF, tag="x", name=f"xv{d}") for d in range(NDT)]
            for d in range(NDT):
                nc.scalar.dma_start(out=wv_t[d], in_=wv[d * BLK:(d + 1) * BLK, :])
                eng = nc.sync if d % 2 == 0 else nc.scalar
                eng.dma_start(out=xv_t[d], in_=xvT[d * BLK:(d + 1) * BLK, :])
            wk_t = [persist.tile([BLK, FB], BF, name=f"wk{d}") for d in range(NDT)]
            xk_t = [xpool.tile([BLK, S], BF, tag="x", name=f"xk{d}") for d in range(NDT)]
            wq_t = [persist.tile([BLK, FB], BF, name=f"wq{d}") for d in range(NDT)]
            xq_t = [xpool.tile([BLK, S], BF, tag="x", name=f"xq{d}") for d in range(NDT)]
            for d in range(NDT):
                nc.scalar.dma_start(out=wk_t[d], in_=wk[d * BLK:(d + 1) * BLK, :])
                eng = nc.sync if d % 2 == 0 else nc.scalar
                eng.dma_start(out=xk_t[d], in_=xkT[d * BLK:(d + 1) * BLK, :])
            for d in range(NDT):
                nc.scalar.dma_start(out=wq_t[d], in_=wq[d * BLK:(d + 1) * BLK, :])
                eng = nc.sync if d % 2 == 0 else nc.scalar
                eng.dma_start(out=xq_t[d], in_=xqT[d * BLK:(d + 1) * BLK, :])
            valid_sb = persist.tile([BLK, nvb * BLK], BF, name="valid_sb")
            for (jt, it), slot in sorted(mixed_slots.items(), key=lambda kv: kv[1]):
                nc.scalar.dma_start(
                    out=valid_sb[:, slot * BLK:(slot + 1) * BLK], in_=validT[slot]
                )
            wo_t = [persist.tile([BLK, D], BF, name=f"wo{p}") for p in range(NPAIR)]
            for p in range(NPAIR):
                nc.scalar.dma_start(out=wo_t[p], in_=wo[p * BLK:(p + 1) * BLK, :])

            qT_sb = [persist.tile([BLK, S], BF, name=f"qT{p}") for p in range(NPAIR)]
            kT_sb = [persist.tile([BLK, S], BF, name=f"kT{p}") for p in range(NPAIR)]
            v_sb = [persist.tile([BLK, HPG * VW], BF, name=f"v{j}") for j in range(NJT)]
            ctxT_sb = [persist.tile([BLK, S], BF, name=f"cT{p}") for p in range(NPAIR)]

            # ---- phase 1: projections ------------------------------------
            for j in range(NJT):
                ps = projp.tile([BLK, IBW], F32, tag="pp", name=f"vps{j}")
                for d in range(NDT):
                    nc.tensor.matmul(
                        ps,
                        xv_t[d][:, j * BLK:(j + 1) * BLK],
                        wv_t[d],
                        start=(d == 0),
                        stop=(d == NDT - 1),
                    )
                dst = v_sb[j].rearrange("p (h w) -> p h w", w=VW)
                nc.vector.tensor_copy(dst[:, :, 0:HD], ps.rearrange("p (h w) -> p h w", w=HD))
                nc.vector.memset(dst[:, :, HD:VW], 1.0)

            def project(p, ib, w_t, x_t, dst_sb, nm):
                ps = projp.tile([BLK, IBW], F32, tag="pp", name=nm)
                for d in range(NDT):
                    nc.tensor.matmul(
                        ps,
                        w_t[d][:, p * BLK:(p + 1) * BLK],
                        x_t[d][:, ib * IBW:(ib + 1) * IBW],
                        start=(d == 0),
                        stop=(d == NDT - 1),
                    )
                nc.vector.tensor_copy(dst_sb[p][:, ib * IBW:(ib + 1) * IBW], ps)



            # ---- phase 2: attention (kT/qT projected per pair just ahead) -
            for p in range(NPAIR):
                for ib in range(NIB):
                    project(p, ib, wk_t, xk_t, kT_sb, f"kps{p}_{ib}")
                for ib in range(NIB):
                    project(p, ib, wq_t, xq_t, qT_sb, f"qps{p}_{ib}")
                for ib in range(NIB):
                    jts = plan[ib]
                    ctx0 = ctxp.tile([VW, IBW], F32, tag="ctx", name=f"c0_{p}_{ib}")
                    ctx1 = ctxp.tile([VW, IBW], F32, tag="ctx", name=f"c1_{p}_{ib}")
                    nj = len(jts)
                    sc_t = {}
                    at_t = {}

                    def emit_scores(e):
                        jt, c0, c1, mixed = jts[e]
                        w = c1 - c0
                        sc = scp.tile([BLK, 2 * IBW], F32, tag="sc", name=f"s{p}_{ib}_{jt}")
                        nc.tensor.matmul(
                            sc[:, c0:c1],
                            kT_sb[p][0:HD, jt * BLK:(jt + 1) * BLK],
                            qT_sb[p][0:HD, ib * IBW + c0:ib * IBW + c1],
                            start=True,
                            stop=True,
                        )
                        nc.tensor.matmul(
                            sc[:, IBW:IBW + w],
                            kT_sb[p][HD:BLK, jt * BLK:(jt + 1) * BLK],
                            qT_sb[p][HD:BLK, ib * IBW + c0:ib * IBW + c1],
                            start=True,
                            stop=True,
                            tile_position=(HD, 0),
                        )
                        sc_t[e] = sc

                    def emit_tail(e):
                        jt, c0, c1, mixed = jts[e]
                        w = c1 - c0
                        sc = sc_t.pop(e)
                        at = atp.tile([BLK, 2 * IBW], BF, tag="at", name=f"a{p}_{ib}_{jt}")
                        nc.scalar.activation(
                            out=at[:, c0:IBW + w],
                            in_=sc[:, c0:IBW + w],
                            func=mybir.ActivationFunctionType.Exp,
                            scale=float(EXP_SCALE),
                        )
                        for k in mixed:
                            slot = mixed_slots[(jt, 4 * ib + k)]
                            vs = valid_sb[:, slot * BLK:(slot + 1) * BLK]
                            nc.vector.tensor_mul(
                                at[:, k * BLK:(k + 1) * BLK],
                                at[:, k * BLK:(k + 1) * BLK],
                                vs,
                            )
                            h1c = IBW + k * BLK - c0
                            nc.vector.tensor_mul(
                                at[:, h1c:h1c + BLK], at[:, h1c:h1c + BLK], vs
                            )
                        vv = v_sb[jt].rearrange("p (h w) -> p h w", w=VW)
                        nc.tensor.matmul(
                            ctx0[:, c0:c1],
                            vv[:, 2 * p, :],
                            at[:, c0:c1],
                            start=(e == 0),
                            stop=(e == nj - 1),
                        )
                        nc.tensor.matmul(
                            ctx1[:, c0:c1],
                            vv[:, 2 * p + 1, :],
                            at[:, IBW:IBW + w],
                            start=(e == 0),
                            stop=(e == nj - 1),
                        )

                    emit_scores(0)
                    for e in range(nj):
                        if e + 1 < nj:
                            emit_scores(e + 1)
                        emit_tail(e)
                    # Evacuate ctx+denominator [65, 512] to SBUF bf16 (frees
                    # the PSUM bank) and ship unnormalized ctx into pair-
                    # stacked ctxT (h1 to partitions 64..127).  The two
                    # denominator rows are DMA-reshaped to [128, 8] so the
                    # DVE reciprocal (~7 cyc/elem/lane) runs on 128 lanes,
                    # then bounced to DRAM and partition-broadcast back; one
                    # in-place multiply normalizes the whole (pair, i-block).
                    stgs = []
                    for h, cps in ((0, ctx0), (1, ctx1)):
                        stg = small.tile([VW, IBW], BF, tag="stg", name=f"st{p}_{ib}_{h}")
                        nc.vector.tensor_copy(stg, cps)
                        nc.sync.dma_start(
                            out=ctxT_sb[p][h * HD:(h + 1) * HD, ib * IBW:(ib + 1) * IBW],
                            in_=stg[0:HD, :],
                        )
                        stgs.append(stg)
                    dd = drp.tile([2, IBW], BF, tag="dd", name=f"dd{p}_{ib}")
                    nc.sync.dma_start(out=dd[0:1, :], in_=stgs[0][HD:VW, :])
                    nc.sync.dma_start(out=dd[1:2, :], in_=stgs[1][HD:VW, :])
                    dsp = small.tile([BLK, 2 * IBW // BLK], BF, tag="dsp", name=f"ds{p}_{ib}")
                    nc.sync.dma_start(
                        out=dsp, in_=dd.rearrange("a (q f) -> (a q) f", q=BLK // 2)
                    )
                    rcp = small.tile([BLK, 2 * IBW // BLK], F32, tag="rcp", name=f"rc{p}_{ib}")
                    nc.vector.reciprocal(out=rcp, in_=dsp)
                    dd2 = drp.tile([2, IBW], F32, tag="dd2", name=f"d2{p}_{ib}")
                    nc.sync.dma_start(
                        out=dd2.rearrange("a (q f) -> (a q) f", q=BLK // 2), in_=rcp
                    )
                    rbc = small.tile([BLK, IBW], F32, tag="rbc", name=f"rb{p}_{ib}")
                    nc.sync.dma_start(
                        out=rbc[0:HD, :], in_=dd2[0:1, :].partition_broadcast(HD)
                    )
                    nc.sync.dma_start(
                        out=rbc[HD:BLK, :], in_=dd2[1:2, :].partition_broadcast(HD)
                    )
                    blk = ctxT_sb[p][:, ib * IBW:(ib + 1) * IBW]
                    nc.vector.tensor_mul(blk, blk, rbc)

            # ---- phase 3: output projection ------------------------------
            for st in range(NST):
                for nb in range(2):
                    po = projp.tile([BLK, IBW], F32, tag="pp", name=f"po{st}_{nb}")
                    for p in range(NPAIR):
                        nc.tensor.matmul(
                            po,
                            ctxT_sb[p][:, st * BLK:(st + 1) * BLK],
                            wo_t[p][:, nb * IBW:(nb + 1) * IBW],
                            start=(p == 0),
                            stop=(p == NPAIR - 1),
                        )
                    ot = small.tile([BLK, IBW], F32, tag="ot", name=f"ot{st}_{nb}")
                    nc.scalar.copy(out=ot, in_=po)
                    nc.scalar.dma_start(
                        out=out[st * BLK:(st + 1) * BLK, nb * IBW:(nb + 1) * IBW],
                        in_=ot,
                    )

    nc.compile()
    return nc


_BUILD_CACHE: dict = {}


def _get_nc(mask: np.ndarray):
    states = classify_mask(mask)
    plan, mixed_slots = build_plan(states)
    key = plan_key(plan, mixed_slots)
    if key not in _BUILD_CACHE:
        _BUILD_CACHE[key] = (build_nc(plan, mixed_slots), plan, mixed_slots)
    return _BUILD_CACHE[key]


def _make_in_maps(xq, xk, xv, mask, W_q, W_k, W_v, W_o, mixed_slots):
    nvb = max(1, len(mixed_slots))
    vt = np.zeros((nvb, BLK, BLK), BF16)
    m = np.asarray(mask)
    for (jt, it), slot in mixed_slots.items():
        vt[slot] = (~m[it * BLK:(it + 1) * BLK, jt * BLK:(jt + 1) * BLK]).T.astype(BF16)
    xT = {}
    for b in range(B):
        xT[b] = tuple(
            np.asarray(x[b]).T.astype(BF16) for x in (xq, xk, xv)
        )
    in_maps = []
    for c in range(8):
        b, g = c // G, c % G
        cols = slice(g * FB, (g + 1) * FB)
        in_maps.append(
            {
                "xqT": xT[b][0],
                "xkT": xT[b][1],
                "xvT": xT[b][2],
                "wq": np.asarray(W_q)[:, cols].astype(BF16),
                "wk": np.asarray(W_k)[:, cols].astype(BF16),
                "wv": np.asarray(W_v)[:, cols].astype(BF16),
                "wo": np.asarray(W_o)[cols, :].astype(BF16),
                "validT": vt,
            }
        )
    return in_maps


PROFILE = False
last_hw_exec_ns = None


def kernel(xq, xk, xv, mask, W_q, W_k, W_v, W_o):
    global last_hw_exec_ns
    from concourse import bass_utils

    nc, plan, mixed_slots = _get_nc(mask)
    in_maps = _make_in_maps(xq, xk, xv, mask, W_q, W_k, W_v, W_o, mixed_slots)
    kwargs = {}
    if PROFILE:
        try:
            import ntff_hook

            if ntff_hook.install():
                import tempfile

                kwargs = {
                    "trace": True,
                    "tmpdir": tempfile.mkdtemp(prefix="mha_trace_"),
                }
        except Exception:
            pass
    res = bass_utils.run_bass_kernel_spmd(
        nc, in_maps, core_ids=list(range(8)), **kwargs
    )
    if res.exec_time_ns:
        last_hw_exec_ns = res.exec_time_ns
    out = np.empty((B, S, D), np.float32)
    for b in range(B):
        out[b] = res.results[2 * b]["out"] + res.results[2 * b + 1]["out"]
    return out

